# revision 21
# baseline (speedup 1.0000x reference)
"""APLoss distributed Bass kernel for 8 TRN2 NeuronCores.

Reference math, restructured with an indicator decomposition:
    sur[i,j] = relu(t)^2,  t = negf_i + y_j,  negf_i = MARGIN - f_i
    relu(t)^2 = t^2 * H,   H = 1[t > 0]
    S_i = sum_j sur = negf_i^2 * A_i + 2*negf_i * B_i + C_i
      where A_i = sum_j H_ij, B_i = sum_j H_ij*y_j, C_i = sum_j H_ij*y_j^2
    T_i = masked version with (Am, Bm, Cm) using weights m_j*[1, y, y^2]
    ua_i = (1-g)*u_all[index_p[i]] + g*S_i/N
    up_i = (1-g)*u_pos[index_p[i]] + g*T_i/N
    loss = sum_i (up_i*S_i - ua_i*T_i) / ua_i^2 / (P*N)

Sharding: rows (positives) split 8 ways, 256 rows/core; y replicated.
Device layout: columns j on partitions (128 j-blocks of 128), rows i on
the free axis (256). Per core:
  DVE  (88 blocks): H = (negf + y_j) > 0           (tensor_scalar add,is_gt)
  ACT  (40 blocks): Hs = Sign(negf + y_j)          (activation, bias=y_j)
  PE: W_b^T @ H_b with W_b = [1, y, y^2, m, m*y, m*y^2] (host-built, bf16),
      4-way column-tiled (tile_position) into psumH/psumS row-groups at
      partitions {0,32,64,96}, accumulated over blocks.
  Sign-block sums corrected on device: H.W = (Hs.W + sum(W))/2 with sum(W)
  over the ACT column range passed as host constants.
  Finalize transposed to [128,2] (rows on partitions) for cheap vector ops;
  per-core scalar partial out; host sums the 8 partials.
"""

import os
import sys

if "/opt/trn_rl_repo" not in sys.path:
    sys.path.insert(0, "/opt/trn_rl_repo")

import ml_dtypes
import numpy as np

import concourse.bass as bass
import concourse.tile as tile
from concourse import bacc, mybir
from concourse import bass_utils
from concourse.masks import make_identity
from concourse.tile_rust import add_dep_helper

N = 16384
P = 2048
N_CORES = 8
PC = P // N_CORES          # rows per core (free dim)
JB = 128                   # j-block size (partitions)
NB = N // JB               # number of j-blocks
NH = PC // JB              # halves of the row range (2)
GAMMA = 0.99
MARGIN = 1.0
INV_PN = 1.0 / (P * N)     # 2^-25, exact

DVE_BLOCKS = 88            # H-blocks on the vector engine; rest on scalar (even)
WK = 16                    # padded stationary columns per block (fp8 DoubleRow
                           # needs 16-byte steps between K-tiles)

TRACE = False
LAST_RESULT = None

_COMPILED = {}

f32 = mybir.dt.float32
bf16 = mybir.dt.bfloat16
f8 = mybir.dt.float8e4
Alu = mybir.AluOpType
Act = mybir.ActivationFunctionType
bfnp = ml_dtypes.bfloat16
f8np = ml_dtypes.float8_e4m3


def _build():
    nc = bacc.Bacc("TRN2", target_bir_lowering=False, debug=False,
                   num_devices=N_CORES)

    # packed inputs (one DMA per dtype keeps descriptor generation short):
    # inf32 = [y(NB) | negfT(2) | uallT(2) | uposT(2) | corrb(12)]
    # inbf  = [negf_bcast(PC) | W(NB*6)]
    F32W = NB + NH + NH + NH + 12
    BFW = PC + NB * 6
    inf32_d = nc.dram_tensor("inf32", [JB, F32W], f32, kind="ExternalInput")
    inbf_d = nc.dram_tensor("inbf", [JB, BFW], bf16, kind="ExternalInput")
    out_d = nc.dram_tensor("out", [1, 1], f32, kind="ExternalOutput")

    with tile.TileContext(nc) as tc:
        with (
            tc.tile_pool(name="const", bufs=1) as cpool,
            tc.tile_pool(name="hpool", bufs=NB) as hpool,
            tc.tile_pool(name="psum", bufs=1, space="PSUM") as ppool,
            tc.tile_pool(name="small", bufs=1) as spool,
        ):
            inf32 = cpool.tile([JB, F32W], f32, name="inf32")
            nc.sync.dma_start(inf32[:], inf32_d[:])
            inbf = cpool.tile([JB, BFW], bf16, name="inbf")
            nc.sync.dma_start(inbf[:], inbf_d[:])
            y_f32 = inf32[:, 0:NB]
            negfT = inf32[:, NB:NB + NH]
            uallT = inf32[:, NB + NH:NB + 2 * NH]
            uposT = inf32[:, NB + 2 * NH:NB + 3 * NH]
            corrb = inf32[:, NB + 3 * NH:NB + 3 * NH + 12]
            negf_bf = inbf[:, 0:PC]
            W_all = inbf[:, PC:PC + NB * 6]

            ident = cpool.tile([JB, JB], f32, name="ident")
            make_identity(nc, ident)
            ones_f = cpool.tile([JB, 1], f32, name="ones_f")
            nc.vector.memset(ones_f[:], 1.0)

            # ---- PE warmup burst: keep HAM at full clock before the
            # real matmul stream begins (runs during the input DMAs) ----
            ones_bf = cpool.tile([JB, 1], bf16, name="ones_bf")
            nc.vector.memset(ones_bf[:], 1.0)
            wtile = cpool.tile([JB, PC], bf16, name="wtile")
            nc.vector.memset(wtile[:], 0.5)
            psumW = ppool.tile([1, PC], f32, name="psumW", tag="pg2")
            for _ in range(24):
                nc.tensor.matmul(psumW[:], ones_bf[:], wtile[:],
                                 start=True, stop=True)

            # ---- H pass ----
            h_tiles = []
            for b in range(NB):
                h = hpool.tile([JB, PC], bf16, name=f"h{b}", tag="h")
                if b < DVE_BLOCKS:
                    nc.vector.tensor_scalar(h[:], negf_bf,
                                            y_f32[:, b:b + 1], 0.0,
                                            Alu.add, Alu.is_gt)
                else:
                    nc.scalar.activation(h[:], negf_bf, Act.Sign,
                                         bias=y_f32[:, b:b + 1])
                h_tiles.append(h)

            # ---- PE contraction (interleave the two sets so the PE can
            # consume ACT-produced blocks during DVE production gaps) ----
            psumH = ppool.tile([JB, PC], f32, name="psumH", tag="pg0")
            psumS = ppool.tile([JB, PC], f32, name="psumS", tag="pg1")
            # merge-sort the two sets by predicted H readiness (DVE
            # ~197ns/block, ACT ~402ns/block) so the PE's strict-FIFO
            # queue never head-blocks on an unproduced tile
            ready = [(197.0 * (b + 1), b) for b in range(DVE_BLOCKS)]
            ready += [(402.0 * (b - DVE_BLOCKS + 1) + 5.0, b)
                      for b in range(DVE_BLOCKS, NB)]
            order = [b for _, b in sorted(ready)]
            first_seen = set()
            last_of = {True: max(b for b in order if b < DVE_BLOCKS),
                       False: max(b for b in order if b >= DVE_BLOCKS)}
            prev_mm = None
            for b in order:
                which = b < DVE_BLOCKS
                acc = psumH if which else psumS
                first = which not in first_seen
                first_seen.add(which)
                last = last_of[which] == b
                mm = nc.tensor.matmul(acc[0:6, :],
                                      W_all[:, b * 6:(b + 1) * 6],
                                      h_tiles[b][:], start=first, stop=last,
                                      skip_group_check=True)
                if prev_mm is not None:
                    add_dep_helper(mm.ins, prev_mm.ins,
                                   reason="keep PE consumption order")
                prev_mm = mm

            Hsb = spool.tile([6, PC], f32, name="Hsb")
            nc.vector.tensor_copy(Hsb[:], psumH[0:6, :])
            Ssb = spool.tile([6, PC], f32, name="Ssb")
            nc.scalar.copy(Ssb[:], psumS[0:6, :])

            psumT = ppool.tile([JB, NH * 6], f32, name="psumT", tag="pg0")
            psumT2 = ppool.tile([JB, NH * 6], f32, name="psumT2", tag="pg1")
            for hh in range(NH):
                nc.tensor.transpose(psumT[:, hh * 6:(hh + 1) * 6],
                                    Hsb[:, hh * JB:(hh + 1) * JB],
                                    ident[0:6, 0:6])
                nc.tensor.transpose(psumT2[:, hh * 6:(hh + 1) * 6],
                                    Ssb[:, hh * JB:(hh + 1) * JB],
                                    ident[0:6, 0:6])

            # finalize on [128, 2] tiles: 256 rows on partitions, both
            # halves as the two free columns
            VH = spool.tile([JB, 12], f32, name="VH")
            nc.vector.tensor_copy(VH[:], psumT[:])
            VS = spool.tile([JB, 12], f32, name="VS")
            nc.vector.tensor_copy(VS[:], psumT2[:])
            Vc = spool.tile([JB, 12], f32, name="Vc")
            nc.vector.tensor_add(Vc[:], VS[:], corrb)
            nc.vector.scalar_tensor_tensor(Vc[:], Vc[:], 0.5, VH[:],
                                           Alu.mult, Alu.add)
            v = Vc[:].rearrange("p (h k) -> p h k", k=6)
            A2, B2, C2 = v[:, :, 0], v[:, :, 1], v[:, :, 2]
            Am2, Bm2, Cm2 = v[:, :, 3], v[:, :, 4], v[:, :, 5]

            nf2 = spool.tile([JB, NH], f32, name="nf2")
            nc.vector.tensor_mul(nf2[:], negfT, negfT)
            n2 = spool.tile([JB, NH], f32, name="n2")
            nc.vector.tensor_scalar(n2[:], negfT, 2.0, 0.0,
                                    Alu.mult, Alu.add)

            S2 = spool.tile([JB, NH], f32, name="S2")
            nc.vector.tensor_mul(S2[:], B2, n2[:])
            nc.vector.tensor_add(S2[:], S2[:], C2)
            t2a = spool.tile([JB, NH], f32, name="t2a")
            nc.vector.tensor_mul(t2a[:], A2, nf2[:])
            nc.vector.tensor_add(S2[:], S2[:], t2a[:])

            T2 = spool.tile([JB, NH], f32, name="T2")
            nc.vector.tensor_mul(T2[:], Bm2, n2[:])
            nc.vector.tensor_add(T2[:], T2[:], Cm2)
            nc.vector.tensor_mul(t2a[:], Am2, nf2[:])
            nc.vector.tensor_add(T2[:], T2[:], t2a[:])

            ua2 = spool.tile([JB, NH], f32, name="ua2")
            nc.vector.tensor_scalar(ua2[:], uallT, 1.0 - GAMMA, 0.0,
                                    Alu.mult, Alu.add)
            nc.vector.scalar_tensor_tensor(ua2[:], S2[:], GAMMA / N, ua2[:],
                                           Alu.mult, Alu.add)
            up2 = spool.tile([JB, NH], f32, name="up2")
            nc.vector.tensor_scalar(up2[:], uposT, 1.0 - GAMMA, 0.0,
                                    Alu.mult, Alu.add)
            nc.vector.scalar_tensor_tensor(up2[:], T2[:], GAMMA / N, up2[:],
                                           Alu.mult, Alu.add)

            inv2 = spool.tile([JB, NH], f32, name="inv2")
            nc.vector.reciprocal(inv2[:], ua2[:])

            d1 = spool.tile([JB, NH], f32, name="d1")
            nc.vector.tensor_mul(d1[:], up2[:], S2[:])
            d2 = spool.tile([JB, NH], f32, name="d2")
            nc.vector.tensor_mul(d2[:], ua2[:], T2[:])
            nc.vector.tensor_sub(d1[:], d1[:], d2[:])
            nc.vector.tensor_mul(d1[:], d1[:], inv2[:])
            nc.vector.tensor_mul(d1[:], d1[:], inv2[:])

            csum = spool.tile([JB, 1], f32, name="csum")
            nc.vector.tensor_add(csum[:], d1[:, 0:1], d1[:, 1:2])
            psum1 = ppool.tile([1, 1], f32, name="psum1", tag="pg2")
            nc.tensor.matmul(psum1[:], ones_f[:], csum[:], start=True,
                             stop=True)
            partial = spool.tile([1, 1], f32, name="partial")
            nc.vector.tensor_scalar(partial[:], psum1[:], INV_PN, 0.0,
                                    Alu.mult, Alu.add)
            nc.sync.dma_start(out_d[:], partial[:])

    nc.compile()
    return nc


def _host_w(yb: np.ndarray, maskb: np.ndarray):
    """W[p, b, :] = [1, y, y^2, m, m*y, m*y^2] in bf16."""
    y = yb.astype(np.float32)
    y2 = (y * y).astype(np.float32)
    m = maskb.astype(np.float32)
    w = np.stack([np.ones_like(y), y, y2, m, m * y, m * y2], axis=-1)
    wb = w.astype(bfnp)
    return np.ascontiguousarray(wb.reshape(JB, NB * 6))


def kernel(y_pred, y_true, index_p, pos_idx, u_all, u_pos):
    global LAST_RESULT

    yp = np.asarray(y_pred, dtype=np.float32).reshape(-1)
    maskf = (np.asarray(y_true, dtype=np.float32).reshape(-1) == 1.0
             ).astype(np.float32)
    index_p = np.asarray(index_p).reshape(-1)
    pos_idx = np.asarray(pos_idx).reshape(-1)
    u_all_b = np.asarray(u_all, dtype=np.float32).reshape(-1)[index_p]
    u_pos_b = np.asarray(u_pos, dtype=np.float32).reshape(-1)[index_p]

    f_ps = yp[pos_idx]
    negf = (MARGIN - f_ps).astype(np.float32)       # (P,)

    nc = _COMPILED.get("nc")
    if nc is None:
        nc = _build()
        _COMPILED["nc"] = nc

    yb = np.ascontiguousarray(yp.reshape(NB, JB).T)
    maskb = np.ascontiguousarray(maskf.reshape(NB, JB).T)
    W = _host_w(yb, maskb)

    # sums of the (bf16-rounded) W columns over the ACT block range, for
    # the sign correction H.W = (Hs.W + sum(W))/2
    Wf = W.reshape(JB, NB, 6).astype(np.float64)
    corr = Wf[:, DVE_BLOCKS:, :].sum(axis=(0, 1)).astype(np.float32)
    corr12 = np.concatenate([corr, corr])  # [A,B,C,Am,Bm,Cm] x 2 halves
    corrb = np.ascontiguousarray(
        np.broadcast_to(corr12, (JB, 12))).astype(np.float32)

    in_maps = []
    for c in range(N_CORES):
        rs = slice(c * PC, (c + 1) * PC)
        negf_c = negf[rs]
        inf32 = np.concatenate([
            yb,
            np.ascontiguousarray(negf_c.reshape(NH, JB).T),
            np.ascontiguousarray(u_all_b[rs].reshape(NH, JB).T),
            np.ascontiguousarray(u_pos_b[rs].reshape(NH, JB).T),
            corrb,
        ], axis=1).astype(np.float32)
        inbf = np.concatenate([
            np.broadcast_to(negf_c, (JB, PC)).astype(bfnp),
            W,
        ], axis=1)
        in_maps.append({
            "inf32": np.ascontiguousarray(inf32),
            "inbf": np.ascontiguousarray(inbf),
        })

    res = None
    last_exc = None
    for attempt in range(2):
        try:
            res = bass_utils.run_bass_kernel_spmd(
                nc, in_maps, core_ids=list(range(N_CORES)), trace=TRACE)
            break
        except Exception as e:
            # the axon/NRT path sporadically reports
            # NRT_EXEC_UNIT_UNRECOVERABLE on an otherwise-good NEFF and
            # the wedge persists within the process; retry once, then
            # fall back to a fresh subprocess (always recovers)
            last_exc = e
            import time
            time.sleep(2.0)
    if res is None:
        if os.environ.get("APLOSS_NO_SUBPROC"):
            raise last_exc
        return _kernel_subprocess(dict(
            y_pred=y_pred, y_true=y_true, index_p=index_p,
            pos_idx=pos_idx, u_all=u_all, u_pos=u_pos))
    LAST_RESULT = res

    total = np.float32(0.0)
    for c in range(N_CORES):
        total = np.float32(total + res.results[c]["out"][0, 0])
    return np.asarray(total, dtype=np.float32)


def _kernel_subprocess(inputs):
    """Run kernel() in a fresh python process (device-wedge recovery)."""
    import subprocess
    import tempfile

    with tempfile.TemporaryDirectory() as td:
        inp = os.path.join(td, "in.npz")
        outp = os.path.join(td, "out.npy")
        np.savez(inp, **{k: np.asarray(v) for k, v in inputs.items()})
        code = (
            "import numpy as np, importlib.util; "
            f"spec = importlib.util.spec_from_file_location('kmod', {__file__!r}); "
            "m = importlib.util.module_from_spec(spec); "
            "spec.loader.exec_module(m); "
            f"d = dict(np.load({inp!r})); "
            f"np.save({outp!r}, m.kernel(**d))"
        )
        env = dict(os.environ, APLOSS_NO_SUBPROC="1")
        subprocess.run([sys.executable, "-c", code], check=True, env=env,
                       timeout=1800)
        return np.load(outp)


# revision 22
# speedup vs baseline: 1.0208x; 1.0208x over previous
"""APLoss distributed Bass kernel for 8 TRN2 NeuronCores.

Reference math, restructured with an indicator decomposition:
    sur[i,j] = relu(t)^2,  t = negf_i + y_j,  negf_i = MARGIN - f_i
    relu(t)^2 = t^2 * H,   H = 1[t > 0]
    S_i = sum_j sur = negf_i^2 * A_i + 2*negf_i * B_i + C_i
      where A_i = sum_j H_ij, B_i = sum_j H_ij*y_j, C_i = sum_j H_ij*y_j^2
    T_i = masked version with (Am, Bm, Cm) using weights m_j*[1, y, y^2]
    ua_i = (1-g)*u_all[index_p[i]] + g*S_i/N
    up_i = (1-g)*u_pos[index_p[i]] + g*T_i/N
    loss = sum_i (up_i*S_i - ua_i*T_i) / ua_i^2 / (P*N)

Sharding: rows (positives) split 8 ways, 256 rows/core; y replicated.
Device layout: columns j on partitions (128 j-blocks of 128), rows i on
the free axis (256). Per core:
  DVE  (88 blocks): H = (negf + y_j) > 0           (tensor_scalar add,is_gt)
  ACT  (40 blocks): Hs = Sign(negf + y_j)          (activation, bias=y_j)
  PE: W_b^T @ H_b with W_b = [1, y, y^2, m, m*y, m*y^2] (host-built, bf16),
      4-way column-tiled (tile_position) into psumH/psumS row-groups at
      partitions {0,32,64,96}, accumulated over blocks.
  Sign-block sums corrected on device: H.W = (Hs.W + sum(W))/2 with sum(W)
  over the ACT column range passed as host constants.
  Finalize transposed to [128,2] (rows on partitions) for cheap vector ops;
  per-core scalar partial out; host sums the 8 partials.
"""

import os
import sys

if "/opt/trn_rl_repo" not in sys.path:
    sys.path.insert(0, "/opt/trn_rl_repo")

import ml_dtypes
import numpy as np

import concourse.bass as bass
import concourse.tile as tile
from concourse import bacc, mybir
from concourse import bass_utils
from concourse.masks import make_identity
from concourse.tile_rust import add_dep_helper

N = 16384
P = 2048
N_CORES = 8
PC = P // N_CORES          # rows per core (free dim)
JB = 128                   # j-block size (partitions)
NB = N // JB               # number of j-blocks
NH = PC // JB              # halves of the row range (2)
GAMMA = 0.99
MARGIN = 1.0
INV_PN = 1.0 / (P * N)     # 2^-25, exact

DVE_BLOCKS = 88            # H-blocks on the vector engine; rest on scalar (even)
WK = 16                    # padded stationary columns per block (fp8 DoubleRow
                           # needs 16-byte steps between K-tiles)

TRACE = False
LAST_RESULT = None

_COMPILED = {}

f32 = mybir.dt.float32
bf16 = mybir.dt.bfloat16
f8 = mybir.dt.float8e4
Alu = mybir.AluOpType
Act = mybir.ActivationFunctionType
bfnp = ml_dtypes.bfloat16
f8np = ml_dtypes.float8_e4m3


def _build():
    nc = bacc.Bacc("TRN2", target_bir_lowering=False, debug=False,
                   num_devices=N_CORES)

    # packed inputs (one DMA per dtype keeps descriptor generation short):
    # inf32 = [y(NB) | negfT(2) | uallT(2) | uposT(2) | corrb(12)]
    # inbf  = [negf_bcast(PC) | W(NB*6)]
    F32W = NB + NH + NH + NH + 12
    BFW = PC + NB * 6
    inf32_d = nc.dram_tensor("inf32", [JB, F32W], f32, kind="ExternalInput")
    inbf_d = nc.dram_tensor("inbf", [JB, BFW], bf16, kind="ExternalInput")
    out_d = nc.dram_tensor("out", [1, 1], f32, kind="ExternalOutput")

    with tile.TileContext(nc) as tc:
        with (
            tc.tile_pool(name="const", bufs=1) as cpool,
            tc.tile_pool(name="hpool", bufs=NB) as hpool,
            tc.tile_pool(name="psum", bufs=1, space="PSUM") as ppool,
            tc.tile_pool(name="small", bufs=1) as spool,
        ):
            inbf = cpool.tile([JB, BFW], bf16, name="inbf")
            nc.gpsimd.dma_start(inbf[:], inbf_d[:])
            inf32 = cpool.tile([JB, F32W], f32, name="inf32")
            nc.sync.dma_start(inf32[:], inf32_d[:])
            y_f32 = inf32[:, 0:NB]
            negfT = inf32[:, NB:NB + NH]
            uallT = inf32[:, NB + NH:NB + 2 * NH]
            uposT = inf32[:, NB + 2 * NH:NB + 3 * NH]
            corrb = inf32[:, NB + 3 * NH:NB + 3 * NH + 12]
            negf_bf = inbf[:, 0:PC]
            W_all = inbf[:, PC:PC + NB * 6]

            ident = cpool.tile([JB, JB], f32, name="ident")
            make_identity(nc, ident)
            ones_f = cpool.tile([JB, 1], f32, name="ones_f")
            nc.vector.memset(ones_f[:], 1.0)

            # ---- PE warmup burst: keep HAM at full clock before the
            # real matmul stream begins (runs during the input DMAs) ----
            ones_bf = cpool.tile([JB, 1], bf16, name="ones_bf")
            nc.vector.memset(ones_bf[:], 1.0)
            wtile = cpool.tile([JB, PC], bf16, name="wtile")
            nc.vector.memset(wtile[:], 0.5)
            psumW = ppool.tile([1, PC], f32, name="psumW", tag="pg2")
            for _ in range(24):
                nc.tensor.matmul(psumW[:], ones_bf[:], wtile[:],
                                 start=True, stop=True)

            # ---- H pass ----
            h_tiles = []
            for b in range(NB):
                h = hpool.tile([JB, PC], bf16, name=f"h{b}", tag="h")
                if b < DVE_BLOCKS:
                    nc.vector.tensor_scalar(h[:], negf_bf,
                                            y_f32[:, b:b + 1], 0.0,
                                            Alu.add, Alu.is_gt)
                else:
                    nc.scalar.activation(h[:], negf_bf, Act.Sign,
                                         bias=y_f32[:, b:b + 1])
                h_tiles.append(h)

            # ---- PE contraction (interleave the two sets so the PE can
            # consume ACT-produced blocks during DVE production gaps) ----
            psumH = ppool.tile([JB, PC], f32, name="psumH", tag="pg0")
            psumS = ppool.tile([JB, PC], f32, name="psumS", tag="pg1")
            # merge-sort the two sets by predicted H readiness (DVE
            # ~197ns/block, ACT ~402ns/block) so the PE's strict-FIFO
            # queue never head-blocks on an unproduced tile
            ready = [(197.0 * (b + 1), b) for b in range(DVE_BLOCKS)]
            ready += [(402.0 * (b - DVE_BLOCKS + 1) + 5.0, b)
                      for b in range(DVE_BLOCKS, NB)]
            order = [b for _, b in sorted(ready)]
            first_seen = set()
            last_of = {True: max(b for b in order if b < DVE_BLOCKS),
                       False: max(b for b in order if b >= DVE_BLOCKS)}
            prev_mm = None
            for b in order:
                which = b < DVE_BLOCKS
                acc = psumH if which else psumS
                first = which not in first_seen
                first_seen.add(which)
                last = last_of[which] == b
                mm = nc.tensor.matmul(acc[0:6, :],
                                      W_all[:, b * 6:(b + 1) * 6],
                                      h_tiles[b][:], start=first, stop=last,
                                      skip_group_check=True)
                if prev_mm is not None:
                    add_dep_helper(mm.ins, prev_mm.ins,
                                   reason="keep PE consumption order")
                prev_mm = mm

            Hsb = spool.tile([6, PC], f32, name="Hsb")
            nc.vector.tensor_copy(Hsb[:], psumH[0:6, :])
            Ssb = spool.tile([6, PC], f32, name="Ssb")
            nc.scalar.copy(Ssb[:], psumS[0:6, :])

            psumT = ppool.tile([JB, NH * 6], f32, name="psumT", tag="pg0")
            psumT2 = ppool.tile([JB, NH * 6], f32, name="psumT2", tag="pg1")
            for hh in range(NH):
                nc.tensor.transpose(psumT[:, hh * 6:(hh + 1) * 6],
                                    Hsb[:, hh * JB:(hh + 1) * JB],
                                    ident[0:6, 0:6])
                nc.tensor.transpose(psumT2[:, hh * 6:(hh + 1) * 6],
                                    Ssb[:, hh * JB:(hh + 1) * JB],
                                    ident[0:6, 0:6])

            # finalize on [128, 2] tiles: 256 rows on partitions, both
            # halves as the two free columns
            VH = spool.tile([JB, 12], f32, name="VH")
            nc.vector.tensor_copy(VH[:], psumT[:])
            VS = spool.tile([JB, 12], f32, name="VS")
            nc.vector.tensor_copy(VS[:], psumT2[:])
            Vc = spool.tile([JB, 12], f32, name="Vc")
            nc.vector.tensor_add(Vc[:], VS[:], corrb)
            nc.vector.scalar_tensor_tensor(Vc[:], Vc[:], 0.5, VH[:],
                                           Alu.mult, Alu.add)
            v = Vc[:].rearrange("p (h k) -> p h k", k=6)
            A2, B2, C2 = v[:, :, 0], v[:, :, 1], v[:, :, 2]
            Am2, Bm2, Cm2 = v[:, :, 3], v[:, :, 4], v[:, :, 5]

            nf2 = spool.tile([JB, NH], f32, name="nf2")
            nc.vector.tensor_mul(nf2[:], negfT, negfT)
            n2 = spool.tile([JB, NH], f32, name="n2")
            nc.vector.tensor_scalar(n2[:], negfT, 2.0, 0.0,
                                    Alu.mult, Alu.add)

            S2 = spool.tile([JB, NH], f32, name="S2")
            nc.vector.tensor_mul(S2[:], B2, n2[:])
            nc.vector.tensor_add(S2[:], S2[:], C2)
            t2a = spool.tile([JB, NH], f32, name="t2a")
            nc.vector.tensor_mul(t2a[:], A2, nf2[:])
            nc.vector.tensor_add(S2[:], S2[:], t2a[:])

            T2 = spool.tile([JB, NH], f32, name="T2")
            nc.vector.tensor_mul(T2[:], Bm2, n2[:])
            nc.vector.tensor_add(T2[:], T2[:], Cm2)
            nc.vector.tensor_mul(t2a[:], Am2, nf2[:])
            nc.vector.tensor_add(T2[:], T2[:], t2a[:])

            # uallT/uposT arrive pre-scaled by (1-GAMMA) from the host
            ua2 = spool.tile([JB, NH], f32, name="ua2")
            nc.vector.scalar_tensor_tensor(ua2[:], S2[:], GAMMA / N, uallT,
                                           Alu.mult, Alu.add)
            up2 = spool.tile([JB, NH], f32, name="up2")
            nc.vector.scalar_tensor_tensor(up2[:], T2[:], GAMMA / N, uposT,
                                           Alu.mult, Alu.add)

            inv2 = spool.tile([JB, NH], f32, name="inv2")
            nc.vector.reciprocal(inv2[:], ua2[:])

            d1 = spool.tile([JB, NH], f32, name="d1")
            nc.vector.tensor_mul(d1[:], up2[:], S2[:])
            d2 = spool.tile([JB, NH], f32, name="d2")
            nc.vector.tensor_mul(d2[:], ua2[:], T2[:])
            nc.vector.tensor_sub(d1[:], d1[:], d2[:])
            nc.vector.tensor_mul(d1[:], d1[:], inv2[:])
            nc.vector.tensor_mul(d1[:], d1[:], inv2[:])

            csum = spool.tile([JB, 1], f32, name="csum")
            nc.vector.tensor_add(csum[:], d1[:, 0:1], d1[:, 1:2])
            psum1 = ppool.tile([1, 1], f32, name="psum1", tag="pg2")
            nc.tensor.matmul(psum1[:], ones_f[:], csum[:], start=True,
                             stop=True)
            partial = spool.tile([1, 1], f32, name="partial")
            nc.vector.tensor_scalar(partial[:], psum1[:], INV_PN, 0.0,
                                    Alu.mult, Alu.add)
            nc.sync.dma_start(out_d[:], partial[:])

    nc.compile()
    return nc


def _host_w(yb: np.ndarray, maskb: np.ndarray):
    """W[p, b, :] = [1, y, y^2, m, m*y, m*y^2] in bf16."""
    y = yb.astype(np.float32)
    y2 = (y * y).astype(np.float32)
    m = maskb.astype(np.float32)
    w = np.stack([np.ones_like(y), y, y2, m, m * y, m * y2], axis=-1)
    wb = w.astype(bfnp)
    return np.ascontiguousarray(wb.reshape(JB, NB * 6))


def kernel(y_pred, y_true, index_p, pos_idx, u_all, u_pos):
    global LAST_RESULT

    yp = np.asarray(y_pred, dtype=np.float32).reshape(-1)
    maskf = (np.asarray(y_true, dtype=np.float32).reshape(-1) == 1.0
             ).astype(np.float32)
    index_p = np.asarray(index_p).reshape(-1)
    pos_idx = np.asarray(pos_idx).reshape(-1)
    u_all_b = np.asarray(u_all, dtype=np.float32).reshape(-1)[index_p]
    u_pos_b = np.asarray(u_pos, dtype=np.float32).reshape(-1)[index_p]

    f_ps = yp[pos_idx]
    negf = (MARGIN - f_ps).astype(np.float32)       # (P,)

    nc = _COMPILED.get("nc")
    if nc is None:
        nc = _build()
        _COMPILED["nc"] = nc

    yb = np.ascontiguousarray(yp.reshape(NB, JB).T)
    maskb = np.ascontiguousarray(maskf.reshape(NB, JB).T)
    W = _host_w(yb, maskb)

    # sums of the (bf16-rounded) W columns over the ACT block range, for
    # the sign correction H.W = (Hs.W + sum(W))/2
    Wf = W.reshape(JB, NB, 6).astype(np.float64)
    corr = Wf[:, DVE_BLOCKS:, :].sum(axis=(0, 1)).astype(np.float32)
    corr12 = np.concatenate([corr, corr])  # [A,B,C,Am,Bm,Cm] x 2 halves
    corrb = np.ascontiguousarray(
        np.broadcast_to(corr12, (JB, 12))).astype(np.float32)

    in_maps = []
    for c in range(N_CORES):
        rs = slice(c * PC, (c + 1) * PC)
        negf_c = negf[rs]
        inf32 = np.concatenate([
            yb,
            np.ascontiguousarray(negf_c.reshape(NH, JB).T),
            np.ascontiguousarray(
                (1.0 - GAMMA) * u_all_b[rs].reshape(NH, JB).T),
            np.ascontiguousarray(
                (1.0 - GAMMA) * u_pos_b[rs].reshape(NH, JB).T),
            corrb,
        ], axis=1).astype(np.float32)
        inbf = np.concatenate([
            np.broadcast_to(negf_c, (JB, PC)).astype(bfnp),
            W,
        ], axis=1)
        in_maps.append({
            "inf32": np.ascontiguousarray(inf32),
            "inbf": np.ascontiguousarray(inbf),
        })

    res = None
    last_exc = None
    for attempt in range(2):
        try:
            res = bass_utils.run_bass_kernel_spmd(
                nc, in_maps, core_ids=list(range(N_CORES)), trace=TRACE)
            break
        except Exception as e:
            # the axon/NRT path sporadically reports
            # NRT_EXEC_UNIT_UNRECOVERABLE on an otherwise-good NEFF and
            # the wedge persists within the process; retry once, then
            # fall back to a fresh subprocess (always recovers)
            last_exc = e
            import time
            time.sleep(2.0)
    if res is None:
        if os.environ.get("APLOSS_NO_SUBPROC"):
            raise last_exc
        return _kernel_subprocess(dict(
            y_pred=y_pred, y_true=y_true, index_p=index_p,
            pos_idx=pos_idx, u_all=u_all, u_pos=u_pos))
    LAST_RESULT = res

    total = np.float32(0.0)
    for c in range(N_CORES):
        total = np.float32(total + res.results[c]["out"][0, 0])
    return np.asarray(total, dtype=np.float32)


def _kernel_subprocess(inputs):
    """Run kernel() in a fresh python process (device-wedge recovery)."""
    import subprocess
    import tempfile

    with tempfile.TemporaryDirectory() as td:
        inp = os.path.join(td, "in.npz")
        outp = os.path.join(td, "out.npy")
        np.savez(inp, **{k: np.asarray(v) for k, v in inputs.items()})
        code = (
            "import numpy as np, importlib.util; "
            f"spec = importlib.util.spec_from_file_location('kmod', {__file__!r}); "
            "m = importlib.util.module_from_spec(spec); "
            "spec.loader.exec_module(m); "
            f"d = dict(np.load({inp!r})); "
            f"np.save({outp!r}, m.kernel(**d))"
        )
        env = dict(os.environ, APLOSS_NO_SUBPROC="1")
        subprocess.run([sys.executable, "-c", code], check=True, env=env,
                       timeout=1800)
        return np.load(outp)


# revision 23
# speedup vs baseline: 1.0374x; 1.0163x over previous
"""APLoss distributed Bass kernel for 8 TRN2 NeuronCores.

Reference math, restructured with an indicator decomposition:
    sur[i,j] = relu(t)^2,  t = negf_i + y_j,  negf_i = MARGIN - f_i
    relu(t)^2 = t^2 * H,   H = 1[t > 0]
    S_i = sum_j sur = negf_i^2 * A_i + 2*negf_i * B_i + C_i
      where A_i = sum_j H_ij, B_i = sum_j H_ij*y_j, C_i = sum_j H_ij*y_j^2
    T_i = masked version with (Am, Bm, Cm) using weights m_j*[1, y, y^2]
    ua_i = (1-g)*u_all[index_p[i]] + g*S_i/N
    up_i = (1-g)*u_pos[index_p[i]] + g*T_i/N
    loss = sum_i (up_i*S_i - ua_i*T_i) / ua_i^2 / (P*N)

Sharding: rows (positives) split 8 ways, 256 rows/core; y replicated.
Device layout: columns j on partitions (128 j-blocks of 128), rows i on
the free axis (256). Per core:
  DVE  (88 blocks): H = (negf + y_j) > 0           (tensor_scalar add,is_gt)
  ACT  (40 blocks): Hs = Sign(negf + y_j)          (activation, bias=y_j)
  PE: W_b^T @ H_b with W_b = [1, y, y^2, m, m*y, m*y^2] (host-built, bf16),
      4-way column-tiled (tile_position) into psumH/psumS row-groups at
      partitions {0,32,64,96}, accumulated over blocks.
  Sign-block sums corrected on device: H.W = (Hs.W + sum(W))/2 with sum(W)
  over the ACT column range passed as host constants.
  Finalize transposed to [128,2] (rows on partitions) for cheap vector ops;
  per-core scalar partial out; host sums the 8 partials.
"""

import os
import sys

if "/opt/trn_rl_repo" not in sys.path:
    sys.path.insert(0, "/opt/trn_rl_repo")

import ml_dtypes
import numpy as np

import concourse.bass as bass
import concourse.tile as tile
from concourse import bacc, mybir
from concourse import bass_utils
from concourse.masks import make_identity
from concourse.tile_rust import add_dep_helper

N = 16384
P = 2048
N_CORES = 8
PC = P // N_CORES          # rows per core (free dim)
JB = 128                   # j-block size (partitions)
NB = N // JB               # number of j-blocks
NH = PC // JB              # halves of the row range (2)
GAMMA = 0.99
MARGIN = 1.0
INV_PN = 1.0 / (P * N)     # 2^-25, exact

DVE_BLOCKS = 88            # H-blocks on the vector engine; rest on scalar (even)
WK = 16                    # padded stationary columns per block (fp8 DoubleRow
                           # needs 16-byte steps between K-tiles)

TRACE = False
LAST_RESULT = None

_COMPILED = {}

f32 = mybir.dt.float32
bf16 = mybir.dt.bfloat16
f8 = mybir.dt.float8e4
Alu = mybir.AluOpType
Act = mybir.ActivationFunctionType
bfnp = ml_dtypes.bfloat16
f8np = ml_dtypes.float8_e4m3


def _build():
    nc = bacc.Bacc("TRN2", target_bir_lowering=False, debug=False,
                   num_devices=N_CORES)

    # packed inputs (one DMA per dtype keeps descriptor generation short):
    # inf32 = [y(NB) | negfT(2) | uallT(2) | uposT(2) | corrb(12)]
    # inbf  = [negf_bcast(PC) | W(NB*6)]
    F32W = NB + NH + NH + NH + 12
    BFW = PC + NB * 6
    inf32_d = nc.dram_tensor("inf32", [JB, F32W], f32, kind="ExternalInput")
    inbf_d = nc.dram_tensor("inbf", [JB, BFW], bf16, kind="ExternalInput")
    out_d = nc.dram_tensor("out", [1, 1], f32, kind="ExternalOutput")

    with tile.TileContext(nc) as tc:
        with (
            tc.tile_pool(name="const", bufs=1) as cpool,
            tc.tile_pool(name="hpool", bufs=NB) as hpool,
            tc.tile_pool(name="psum", bufs=1, space="PSUM") as ppool,
            tc.tile_pool(name="small", bufs=1) as spool,
        ):
            inbf = cpool.tile([JB, BFW], bf16, name="inbf")
            dma_bf = nc.gpsimd.dma_start(inbf[:], inbf_d[:])
            inf32 = cpool.tile([JB, F32W], f32, name="inf32")
            nc.sync.dma_start(inf32[:], inf32_d[:])
            y_f32 = inf32[:, 0:NB]
            negfT = inf32[:, NB:NB + NH]
            uallT = inf32[:, NB + NH:NB + 2 * NH]
            uposT = inf32[:, NB + 2 * NH:NB + 3 * NH]
            corrb = inf32[:, NB + 3 * NH:NB + 3 * NH + 12]
            negf_bf = inbf[:, 0:PC]
            W_all = inbf[:, PC:PC + NB * 6]

            # identity build is gpsimd work; order it behind the input DMA
            # trigger so the DMA descriptor walk starts immediately
            ident = cpool.tile([JB, JB], f32, name="ident")
            mz = nc.gpsimd.memset(ident[:], 0.0)
            add_dep_helper(mz.ins, dma_bf.ins, reason="dma trigger first")
            nc.gpsimd.affine_select(
                out=ident[:], in_=ident[:],
                compare_op=Alu.not_equal, fill=1.0, base=0,
                pattern=[[-1, JB]], channel_multiplier=1)
            ones_f = cpool.tile([JB, 1], f32, name="ones_f")
            nc.vector.memset(ones_f[:], 1.0)

            # ---- PE warmup burst: keep HAM at full clock before the
            # real matmul stream begins (runs during the input DMAs) ----
            ones_bf = cpool.tile([JB, 1], bf16, name="ones_bf")
            nc.vector.memset(ones_bf[:], 1.0)
            wtile = cpool.tile([JB, PC], bf16, name="wtile")
            nc.vector.memset(wtile[:], 0.5)
            psumW = ppool.tile([1, PC], f32, name="psumW", tag="pg2")
            for _ in range(24):
                nc.tensor.matmul(psumW[:], ones_bf[:], wtile[:],
                                 start=True, stop=True)

            # ---- H pass ----
            h_tiles = []
            for b in range(NB):
                h = hpool.tile([JB, PC], bf16, name=f"h{b}", tag="h")
                if b < DVE_BLOCKS:
                    nc.vector.tensor_scalar(h[:], negf_bf,
                                            y_f32[:, b:b + 1], 0.0,
                                            Alu.add, Alu.is_gt)
                else:
                    nc.scalar.activation(h[:], negf_bf, Act.Sign,
                                         bias=y_f32[:, b:b + 1])
                h_tiles.append(h)

            # ---- PE contraction (interleave the two sets so the PE can
            # consume ACT-produced blocks during DVE production gaps) ----
            psumH = ppool.tile([JB, PC], f32, name="psumH", tag="pg0")
            psumS = ppool.tile([JB, PC], f32, name="psumS", tag="pg1")
            # merge-sort the two sets by predicted H readiness (DVE
            # ~197ns/block, ACT ~402ns/block) so the PE's strict-FIFO
            # queue never head-blocks on an unproduced tile
            ready = [(197.0 * (b + 1), b) for b in range(DVE_BLOCKS)]
            ready += [(402.0 * (b - DVE_BLOCKS + 1) + 5.0, b)
                      for b in range(DVE_BLOCKS, NB)]
            order = [b for _, b in sorted(ready)]
            first_seen = set()
            last_of = {True: max(b for b in order if b < DVE_BLOCKS),
                       False: max(b for b in order if b >= DVE_BLOCKS)}
            prev_mm = None
            for b in order:
                which = b < DVE_BLOCKS
                acc = psumH if which else psumS
                first = which not in first_seen
                first_seen.add(which)
                last = last_of[which] == b
                mm = nc.tensor.matmul(acc[0:6, :],
                                      W_all[:, b * 6:(b + 1) * 6],
                                      h_tiles[b][:], start=first, stop=last,
                                      skip_group_check=True)
                if prev_mm is not None:
                    add_dep_helper(mm.ins, prev_mm.ins,
                                   reason="keep PE consumption order")
                prev_mm = mm

            Hsb = spool.tile([6, PC], f32, name="Hsb")
            nc.vector.tensor_copy(Hsb[:], psumH[0:6, :])
            Ssb = spool.tile([6, PC], f32, name="Ssb")
            nc.scalar.copy(Ssb[:], psumS[0:6, :])

            psumT = ppool.tile([JB, NH * 6], f32, name="psumT", tag="pg0")
            psumT2 = ppool.tile([JB, NH * 6], f32, name="psumT2", tag="pg1")
            for hh in range(NH):
                nc.tensor.transpose(psumT[:, hh * 6:(hh + 1) * 6],
                                    Hsb[:, hh * JB:(hh + 1) * JB],
                                    ident[0:6, 0:6])
                nc.tensor.transpose(psumT2[:, hh * 6:(hh + 1) * 6],
                                    Ssb[:, hh * JB:(hh + 1) * JB],
                                    ident[0:6, 0:6])

            # finalize on [128, 2] tiles: 256 rows on partitions, both
            # halves as the two free columns
            VH = spool.tile([JB, 12], f32, name="VH")
            nc.vector.tensor_copy(VH[:], psumT[:])
            VS = spool.tile([JB, 12], f32, name="VS")
            nc.vector.tensor_copy(VS[:], psumT2[:])
            Vc = spool.tile([JB, 12], f32, name="Vc")
            nc.vector.tensor_add(Vc[:], VS[:], corrb)
            nc.vector.scalar_tensor_tensor(Vc[:], Vc[:], 0.5, VH[:],
                                           Alu.mult, Alu.add)
            v = Vc[:].rearrange("p (h k) -> p h k", k=6)
            A2, B2, C2 = v[:, :, 0], v[:, :, 1], v[:, :, 2]
            Am2, Bm2, Cm2 = v[:, :, 3], v[:, :, 4], v[:, :, 5]

            nf2 = spool.tile([JB, NH], f32, name="nf2")
            nc.vector.tensor_mul(nf2[:], negfT, negfT)
            n2 = spool.tile([JB, NH], f32, name="n2")
            nc.vector.tensor_scalar(n2[:], negfT, 2.0, 0.0,
                                    Alu.mult, Alu.add)

            S2 = spool.tile([JB, NH], f32, name="S2")
            nc.vector.tensor_mul(S2[:], B2, n2[:])
            nc.vector.tensor_add(S2[:], S2[:], C2)
            t2a = spool.tile([JB, NH], f32, name="t2a")
            nc.vector.tensor_mul(t2a[:], A2, nf2[:])
            nc.vector.tensor_add(S2[:], S2[:], t2a[:])

            T2 = spool.tile([JB, NH], f32, name="T2")
            nc.vector.tensor_mul(T2[:], Bm2, n2[:])
            nc.vector.tensor_add(T2[:], T2[:], Cm2)
            nc.vector.tensor_mul(t2a[:], Am2, nf2[:])
            nc.vector.tensor_add(T2[:], T2[:], t2a[:])

            # uallT/uposT arrive pre-scaled by (1-GAMMA) from the host
            ua2 = spool.tile([JB, NH], f32, name="ua2")
            nc.vector.scalar_tensor_tensor(ua2[:], S2[:], GAMMA / N, uallT,
                                           Alu.mult, Alu.add)
            up2 = spool.tile([JB, NH], f32, name="up2")
            nc.vector.scalar_tensor_tensor(up2[:], T2[:], GAMMA / N, uposT,
                                           Alu.mult, Alu.add)

            inv2 = spool.tile([JB, NH], f32, name="inv2")
            nc.vector.reciprocal(inv2[:], ua2[:])

            d1 = spool.tile([JB, NH], f32, name="d1")
            nc.vector.tensor_mul(d1[:], up2[:], S2[:])
            d2 = spool.tile([JB, NH], f32, name="d2")
            nc.vector.tensor_mul(d2[:], ua2[:], T2[:])
            nc.vector.tensor_sub(d1[:], d1[:], d2[:])
            nc.vector.tensor_mul(d1[:], d1[:], inv2[:])
            nc.vector.tensor_mul(d1[:], d1[:], inv2[:])

            csum = spool.tile([JB, 1], f32, name="csum")
            nc.vector.tensor_add(csum[:], d1[:, 0:1], d1[:, 1:2])
            psum1 = ppool.tile([1, 1], f32, name="psum1", tag="pg2")
            nc.tensor.matmul(psum1[:], ones_f[:], csum[:], start=True,
                             stop=True)
            partial = spool.tile([1, 1], f32, name="partial")
            nc.vector.tensor_scalar(partial[:], psum1[:], INV_PN, 0.0,
                                    Alu.mult, Alu.add)
            nc.sync.dma_start(out_d[:], partial[:])

    nc.compile()
    return nc


def _host_w(yb: np.ndarray, maskb: np.ndarray):
    """W[p, b, :] = [1, y, y^2, m, m*y, m*y^2] in bf16."""
    y = yb.astype(np.float32)
    y2 = (y * y).astype(np.float32)
    m = maskb.astype(np.float32)
    w = np.stack([np.ones_like(y), y, y2, m, m * y, m * y2], axis=-1)
    wb = w.astype(bfnp)
    return np.ascontiguousarray(wb.reshape(JB, NB * 6))


def kernel(y_pred, y_true, index_p, pos_idx, u_all, u_pos):
    global LAST_RESULT

    yp = np.asarray(y_pred, dtype=np.float32).reshape(-1)
    maskf = (np.asarray(y_true, dtype=np.float32).reshape(-1) == 1.0
             ).astype(np.float32)
    index_p = np.asarray(index_p).reshape(-1)
    pos_idx = np.asarray(pos_idx).reshape(-1)
    u_all_b = np.asarray(u_all, dtype=np.float32).reshape(-1)[index_p]
    u_pos_b = np.asarray(u_pos, dtype=np.float32).reshape(-1)[index_p]

    f_ps = yp[pos_idx]
    negf = (MARGIN - f_ps).astype(np.float32)       # (P,)

    nc = _COMPILED.get("nc")
    if nc is None:
        nc = _build()
        _COMPILED["nc"] = nc

    yb = np.ascontiguousarray(yp.reshape(NB, JB).T)
    maskb = np.ascontiguousarray(maskf.reshape(NB, JB).T)
    W = _host_w(yb, maskb)

    # sums of the (bf16-rounded) W columns over the ACT block range, for
    # the sign correction H.W = (Hs.W + sum(W))/2
    Wf = W.reshape(JB, NB, 6).astype(np.float64)
    corr = Wf[:, DVE_BLOCKS:, :].sum(axis=(0, 1)).astype(np.float32)
    corr12 = np.concatenate([corr, corr])  # [A,B,C,Am,Bm,Cm] x 2 halves
    corrb = np.ascontiguousarray(
        np.broadcast_to(corr12, (JB, 12))).astype(np.float32)

    in_maps = []
    for c in range(N_CORES):
        rs = slice(c * PC, (c + 1) * PC)
        negf_c = negf[rs]
        inf32 = np.concatenate([
            yb,
            np.ascontiguousarray(negf_c.reshape(NH, JB).T),
            np.ascontiguousarray(
                (1.0 - GAMMA) * u_all_b[rs].reshape(NH, JB).T),
            np.ascontiguousarray(
                (1.0 - GAMMA) * u_pos_b[rs].reshape(NH, JB).T),
            corrb,
        ], axis=1).astype(np.float32)
        inbf = np.concatenate([
            np.broadcast_to(negf_c, (JB, PC)).astype(bfnp),
            W,
        ], axis=1)
        in_maps.append({
            "inf32": np.ascontiguousarray(inf32),
            "inbf": np.ascontiguousarray(inbf),
        })

    res = None
    last_exc = None
    for attempt in range(2):
        try:
            res = bass_utils.run_bass_kernel_spmd(
                nc, in_maps, core_ids=list(range(N_CORES)), trace=TRACE)
            break
        except Exception as e:
            # the axon/NRT path sporadically reports
            # NRT_EXEC_UNIT_UNRECOVERABLE on an otherwise-good NEFF and
            # the wedge persists within the process; retry once, then
            # fall back to a fresh subprocess (always recovers)
            last_exc = e
            import time
            time.sleep(2.0)
    if res is None:
        if os.environ.get("APLOSS_NO_SUBPROC"):
            raise last_exc
        return _kernel_subprocess(dict(
            y_pred=y_pred, y_true=y_true, index_p=index_p,
            pos_idx=pos_idx, u_all=u_all, u_pos=u_pos))
    LAST_RESULT = res

    total = np.float32(0.0)
    for c in range(N_CORES):
        total = np.float32(total + res.results[c]["out"][0, 0])
    return np.asarray(total, dtype=np.float32)


def _kernel_subprocess(inputs):
    """Run kernel() in a fresh python process (device-wedge recovery)."""
    import subprocess
    import tempfile

    with tempfile.TemporaryDirectory() as td:
        inp = os.path.join(td, "in.npz")
        outp = os.path.join(td, "out.npy")
        np.savez(inp, **{k: np.asarray(v) for k, v in inputs.items()})
        code = (
            "import numpy as np, importlib.util; "
            f"spec = importlib.util.spec_from_file_location('kmod', {__file__!r}); "
            "m = importlib.util.module_from_spec(spec); "
            "spec.loader.exec_module(m); "
            f"d = dict(np.load({inp!r})); "
            f"np.save({outp!r}, m.kernel(**d))"
        )
        env = dict(os.environ, APLOSS_NO_SUBPROC="1")
        subprocess.run([sys.executable, "-c", code], check=True, env=env,
                       timeout=1800)
        return np.load(outp)
